# revision 17
# baseline (speedup 1.0000x reference)
"""Chamfer distance (weighted, fwd+bwd, mean reduction) on 8 TRN2 NeuronCores.

Math: for pred P[b] (N=8192 x 3) and target T[b] (M=8192 x 3),
  sq(n, m) = |p_n - t_m|^2 = -2 * (p_n . t_m - |p_n|^2/2 - |t_m|^2/2)
One augmented matmul produces out(n, m) = p.t - |p|^2/2 - |t|^2/2 = -sq/2
(all <= 0); then min_m sq = -2 * max_m out (sqrt is monotone, applied on host).

The matmul runs in fp16 at full PE rate (fp32 matmuls cost 4 cycles/row) with
a hi/lo split-precision expansion that recovers fp32-level accuracy:
  p.t = ph.th + pl.th + ph.tl   (pl.tl ~ 2^-22, dropped)
  norms are split the same way, multiplied by a ones-row.
K = 3*3 + 2 + 2 = 13 contraction rows; PE cost is K-independent.
Verified on HW: bit-identical max-results vs the fp32 matmul path.

Sharding: batch b -> core pair (2b, 2b+1); each core takes half the pred rows
(4096) and all 8192 targets. Forward mins are complete per core; backward
partial maxes (over the core's pred rows) are combined on host.

Per core: 32 pred-tiles of 128 rows. Per tile: 16 matmuls (N=512) fill 4 PSUM
slabs [128, 2048]; ScalarE converts each slab f32->fp16 into a [128, 8192]
stage; VectorE does the backward running max (one fp16 2x tensor_tensor) and
the forward max via a fold tree + final 1x reduce.
"""

import numpy as np

import concourse.bacc as bacc
import concourse.mybir as mybir
import concourse.tile as tile
from concourse.bass_utils import run_bass_kernel_spmd

B = 4
N = 8192  # pred points per batch
M = 8192  # target points per batch
D = 3
K = 13  # augmented contraction dim (split precision)
NH = N // 2  # pred rows per core
P = 128  # partitions
NT = NH // P  # pred tiles per core (32)
SLAB = 2048  # psum slab width (4 banks)
NSLAB = M // SLAB  # 4
MM = 512  # matmul free dim (1 psum bank of f32)
N_CORES = 8
EPS = 1e-12

_cached_nc = None


def _build_nc():
    f32 = mybir.dt.float32
    f16 = mybir.dt.float16
    alu_max = mybir.AluOpType.max

    nc = bacc.Bacc("TRN2", target_bir_lowering=False, debug=False)
    paug = nc.dram_tensor("paug", [K, NH], f16, kind="ExternalInput")
    taug = nc.dram_tensor("taug", [K, M], f16, kind="ExternalInput")
    NG = 4  # PE row-group tiles (32-partition strips at 0/32/64/96)
    # fwd_out[p, t] = max over all targets for pred row t*128+p
    fwd_out = nc.dram_tensor("fwd_out", [P, NT], f16, kind="ExternalOutput")
    # bwd_out[p, m] = max over this core's pred rows congruent to p (mod 128)
    bwd_out = nc.dram_tensor("bwd_out", [P, M], f16, kind="ExternalOutput")

    with tile.TileContext(nc) as tc:
        with (
            tc.tile_pool(name="const", bufs=1) as cpool,
            tc.tile_pool(name="stage", bufs=3) as spool,
            tc.tile_pool(name="accp", bufs=2) as apool,
            tc.tile_pool(name="scratch", bufs=2) as zpool,
            tc.tile_pool(name="psum", bufs=2, space="PSUM") as ppool,
        ):
            # Operands replicated into 4 32-partition strips so matmuls can be
            # issued to distinct PE row-groups (tile_position) and overlap.
            # Chunked so the first pred-tile's operands land first and the
            # pipeline fills early.
            taug_sb = cpool.tile([P, M], f16)
            paug_sb = cpool.tile([P, NH], f16)
            # Small first-needed chunks first (DMA issue costs ~770ns each on
            # a single sequencer); bulk remainder spread over idle queues.
            for g in range(NG):
                nc.sync.dma_start(
                    paug_sb[32 * g : 32 * g + K, :P], paug[:, :P]
                )
                nc.sync.dma_start(
                    taug_sb[32 * g : 32 * g + K, :SLAB], taug[:, :SLAB]
                )
            for g in range(NG):
                nc.sync.dma_start(
                    taug_sb[32 * g : 32 * g + K, SLAB:], taug[:, SLAB:]
                )
                nc.gpsimd.dma_start(
                    paug_sb[32 * g : 32 * g + K, P:], paug[:, P:]
                )
            fwd_sb = cpool.tile([P, NT], f16)

            acc = None
            for t in range(NT):
                st = spool.tile([P, M], f16, tag="st")
                for s in range(NSLAB):
                    ps = ppool.tile([P, SLAB], f32, tag="ps")
                    for j in range(SLAB // MM):
                        col = s * SLAB + j * MM
                        g = 32 * (j % NG)
                        nc.tensor.matmul(
                            ps[:, j * MM : (j + 1) * MM],
                            paug_sb[g : g + K, t * P : (t + 1) * P],
                            taug_sb[g : g + K, col : col + MM],
                            start=True,
                            stop=True,
                            tile_position=(g, 0),
                        )
                    # f32 PSUM -> fp16 SBUF stage slab
                    nc.scalar.copy(st[:, s * SLAB : (s + 1) * SLAB], ps[:])
                # backward running max (ping-pong; fp16 tensor_tensor = 2x mode)
                na = apool.tile([P, M], f16, tag="acc")
                if t == 0:
                    nc.vector.tensor_copy(na[:], st[:])
                else:
                    nc.vector.tensor_tensor(na[:], acc[:], st[:], op=alu_max)
                acc = na
                # forward max: fold tree (tensor_reduce is 1x-only, so shrink first)
                f1 = zpool.tile([P, M // 2], f16, tag="f1")
                nc.vector.tensor_tensor(
                    f1[:], st[:, : M // 2], st[:, M // 2 :], op=alu_max
                )
                f2 = zpool.tile([P, M // 4], f16, tag="f2")
                nc.vector.tensor_tensor(
                    f2[:], f1[:, : M // 4], f1[:, M // 4 :], op=alu_max
                )
                f3 = zpool.tile([P, M // 8], f16, tag="f3")
                nc.vector.tensor_tensor(
                    f3[:], f2[:, : M // 8], f2[:, M // 8 :], op=alu_max
                )
                f4 = zpool.tile([P, M // 16], f16, tag="f4")
                nc.vector.tensor_tensor(
                    f4[:], f3[:, : M // 16], f3[:, M // 16 :], op=alu_max
                )
                f5 = zpool.tile([P, M // 32], f16, tag="f5")
                nc.vector.tensor_tensor(
                    f5[:], f4[:, : M // 32], f4[:, M // 32 :], op=alu_max
                )
                nc.vector.reduce_max(
                    fwd_sb[:, t : t + 1], f5[:], axis=mybir.AxisListType.X
                )
            # split the 2MB result across DMA queues
            for s in range(NSLAB):
                nc.sync.dma_start(
                    bwd_out[:, s * SLAB : (s + 1) * SLAB],
                    acc[:, s * SLAB : (s + 1) * SLAB],
                )
            nc.sync.dma_start(fwd_out[:], fwd_sb[:])
    nc.compile()
    return nc


def _get_nc():
    global _cached_nc
    if _cached_nc is None:
        _cached_nc = _build_nc()
    return _cached_nc


def _split16(x):
    """x (f32) -> (hi, lo) fp16 pair with hi + lo ~= x."""
    hi = x.astype(np.float16)
    lo = (x - hi.astype(np.float32)).astype(np.float16)
    return hi, lo


def _make_in_maps(pred, target):
    in_maps = []
    for c in range(N_CORES):
        b, h = divmod(c, 2)
        p = pred[b, h * NH : (h + 1) * NH]  # [4096, 3]
        t = target[b]  # [8192, 3]
        pn = -0.5 * (p * p).sum(-1, dtype=np.float32)
        tn = -0.5 * (t * t).sum(-1, dtype=np.float32)
        ph, pl = _split16(p.T)
        th, tl = _split16(t.T)
        pnh, pnl = _split16(pn)
        tnh, tnl = _split16(tn)
        paug = np.zeros((K, NH), np.float16)
        taug = np.zeros((K, M), np.float16)
        # p.t = ph.th + pl.th + ph.tl ; norms via ones-rows
        paug[0:3] = ph
        paug[3:6] = pl
        paug[6:9] = ph
        paug[9] = pnh
        paug[10] = pnl
        paug[11] = 1.0
        paug[12] = 1.0
        taug[0:3] = th
        taug[3:6] = th
        taug[6:9] = tl
        taug[9] = 1.0
        taug[10] = 1.0
        taug[11] = tnh
        taug[12] = tnl
        in_maps.append({"paug": paug, "taug": taug})
    return in_maps


def _reduce_outputs(results):
    total = 0.0
    for b in range(B):
        fwd_rows = []
        bwd_parts = []
        for h in range(2):
            r = results[2 * b + h]
            fwd = np.asarray(r["fwd_out"], np.float64)  # [128, 32]
            fwd_rows.append(fwd.T.reshape(-1))  # row order n = t*128 + p
            bwd_parts.append(np.asarray(r["bwd_out"], np.float64).max(0))
        fwd_max = np.concatenate(fwd_rows)  # [8192]
        bwd_max = np.maximum(bwd_parts[0], bwd_parts[1])  # [8192]
        fwd_sq = np.maximum(-2.0 * fwd_max, EPS)
        bwd_sq = np.maximum(-2.0 * bwd_max, EPS)
        total += np.sqrt(fwd_sq).sum() + np.sqrt(bwd_sq).sum()
    return np.asarray(total / B, dtype=np.float32)


def kernel(pred, target):
    pred = np.ascontiguousarray(np.asarray(pred, dtype=np.float32))
    target = np.ascontiguousarray(np.asarray(target, dtype=np.float32))
    assert pred.shape == (B, N, D) and target.shape == (B, M, D)
    nc = _get_nc()
    in_maps = _make_in_maps(pred, target)
    res = run_bass_kernel_spmd(nc, in_maps, list(range(N_CORES)))
    return _reduce_outputs(res.results)


# revision 18
# speedup vs baseline: 1.1966x; 1.1966x over previous
"""Chamfer distance (weighted, fwd+bwd, mean reduction) on 8 TRN2 NeuronCores.

Math: for pred P[b] (N=8192 x 3) and target T[b] (M=8192 x 3),
  sq(n, m) = |p_n - t_m|^2 = -2 * (p_n . t_m - |p_n|^2/2 - |t_m|^2/2)
One augmented matmul produces out(n, m) = p.t - |p|^2/2 - |t|^2/2 = -sq/2
(all <= 0); then min_m sq = -2 * max_m out (sqrt is monotone, applied on host).

The matmul runs in fp16 at full PE rate (fp32 matmuls cost 4 cycles/row) with
a hi/lo split-precision expansion that recovers fp32-level accuracy:
  p.t = ph.th + pl.th + ph.tl   (pl.tl ~ 2^-22, dropped)
  norms are split the same way, multiplied by a ones-row.
K = 3*3 + 2 + 2 = 13 contraction rows; PE cost is K-independent.
Verified on HW: bit-identical max-results vs the fp32 matmul path.

Sharding: batch b -> core pair (2b, 2b+1); each core takes half the pred rows
(4096) and all 8192 targets. Forward mins are complete per core; backward
partial maxes (over the core's pred rows) are combined on host.

Per core: 32 pred-tiles of 128 rows. Per tile: 16 matmuls (N=512) fill 4 PSUM
slabs [128, 2048]; ScalarE converts each slab f32->fp16 into a [128, 8192]
stage; VectorE does the backward running max (one fp16 2x tensor_tensor) and
the forward max via a fold tree + final 1x reduce.
"""

import numpy as np

import concourse.bacc as bacc
import concourse.mybir as mybir
import concourse.tile as tile
from concourse.bass_utils import run_bass_kernel_spmd

B = 4
N = 8192  # pred points per batch
M = 8192  # target points per batch
D = 3
K = 13  # augmented contraction dim (split precision)
NH = N // 2  # pred rows per core
P = 128  # partitions
NT = NH // P  # pred tiles per core (32)
SLAB = 2048  # psum slab width (4 banks)
NSLAB = M // SLAB  # 4
MM = 512  # matmul free dim (1 psum bank of f32)
N_CORES = 8
EPS = 1e-12

_cached_nc = None


def _build_nc():
    f32 = mybir.dt.float32
    f16 = mybir.dt.float16
    alu_max = mybir.AluOpType.max

    nc = bacc.Bacc("TRN2", target_bir_lowering=False, debug=False)
    paug = nc.dram_tensor("paug", [K, NH], f16, kind="ExternalInput")
    taug = nc.dram_tensor("taug", [K, M], f16, kind="ExternalInput")
    NG = 4  # PE row-group tiles (32-partition strips at 0/32/64/96)
    # fwd_out[p, t] = max over all targets for pred row t*128+p
    fwd_out = nc.dram_tensor("fwd_out", [P, NT], f16, kind="ExternalOutput")
    # bwd_out[p, m] = max over this core's pred rows congruent to p (mod 128)
    bwd_out = nc.dram_tensor("bwd_out", [P, M], f16, kind="ExternalOutput")

    with tile.TileContext(nc) as tc:
        with (
            tc.tile_pool(name="const", bufs=1) as cpool,
            tc.tile_pool(name="stage", bufs=3) as spool,
            tc.tile_pool(name="accp", bufs=2) as apool,
            tc.tile_pool(name="scratch", bufs=2) as zpool,
            tc.tile_pool(name="psum", bufs=2, space="PSUM") as ppool,
        ):
            # Operands replicated into 4 32-partition strips so matmuls can be
            # issued to distinct PE row-groups (tile_position) and overlap.
            # Chunked so the first pred-tile's operands land first and the
            # pipeline fills early.
            taug_sb = cpool.tile([P, M], f16)
            paug_sb = cpool.tile([P, NH], f16)
            # Small first-needed chunks first (DMA issue costs ~770ns each on
            # a single sequencer); bulk remainder spread over idle queues.
            for g in range(NG):
                nc.sync.dma_start(
                    paug_sb[32 * g : 32 * g + K, :P], paug[:, :P]
                )
                nc.sync.dma_start(
                    taug_sb[32 * g : 32 * g + K, :SLAB], taug[:, :SLAB]
                )
            for g in range(NG):
                nc.sync.dma_start(
                    taug_sb[32 * g : 32 * g + K, SLAB:], taug[:, SLAB:]
                )
                nc.scalar.dma_start(
                    paug_sb[32 * g : 32 * g + K, P:], paug[:, P:]
                )
            fwd_sb = cpool.tile([P, NT], f16)

            acc = None
            for t in range(NT):
                st = spool.tile([P, M], f16, tag="st")
                for s in range(NSLAB):
                    ps = ppool.tile([P, SLAB], f32, tag="ps")
                    for j in range(SLAB // MM):
                        col = s * SLAB + j * MM
                        g = 32 * (j % NG)
                        nc.tensor.matmul(
                            ps[:, j * MM : (j + 1) * MM],
                            paug_sb[g : g + K, t * P : (t + 1) * P],
                            taug_sb[g : g + K, col : col + MM],
                            start=True,
                            stop=True,
                            tile_position=(g, 0),
                        )
                    # f32 PSUM -> fp16 SBUF stage slab
                    nc.scalar.copy(st[:, s * SLAB : (s + 1) * SLAB], ps[:])
                # backward running max (ping-pong; fp16 tensor_tensor = 2x mode)
                na = apool.tile([P, M], f16, tag="acc")
                if t == 0:
                    nc.vector.tensor_copy(na[:], st[:])
                else:
                    nc.vector.tensor_tensor(na[:], acc[:], st[:], op=alu_max)
                acc = na
                # forward max: fold tree (tensor_reduce is 1x-only, so shrink first)
                f1 = zpool.tile([P, M // 2], f16, tag="f1")
                nc.vector.tensor_tensor(
                    f1[:], st[:, : M // 2], st[:, M // 2 :], op=alu_max
                )
                f2 = zpool.tile([P, M // 4], f16, tag="f2")
                nc.vector.tensor_tensor(
                    f2[:], f1[:, : M // 4], f1[:, M // 4 :], op=alu_max
                )
                f3 = zpool.tile([P, M // 8], f16, tag="f3")
                nc.vector.tensor_tensor(
                    f3[:], f2[:, : M // 8], f2[:, M // 8 :], op=alu_max
                )
                f4 = zpool.tile([P, M // 16], f16, tag="f4")
                nc.vector.tensor_tensor(
                    f4[:], f3[:, : M // 16], f3[:, M // 16 :], op=alu_max
                )
                f5 = zpool.tile([P, M // 32], f16, tag="f5")
                nc.vector.tensor_tensor(
                    f5[:], f4[:, : M // 32], f4[:, M // 32 :], op=alu_max
                )
                nc.vector.reduce_max(
                    fwd_sb[:, t : t + 1], f5[:], axis=mybir.AxisListType.X
                )
            # split the 2MB result across DMA queues
            for s in range(NSLAB):
                nc.sync.dma_start(
                    bwd_out[:, s * SLAB : (s + 1) * SLAB],
                    acc[:, s * SLAB : (s + 1) * SLAB],
                )
            nc.sync.dma_start(fwd_out[:], fwd_sb[:])
    nc.compile()
    return nc


def _get_nc():
    global _cached_nc
    if _cached_nc is None:
        _cached_nc = _build_nc()
    return _cached_nc


def _split16(x):
    """x (f32) -> (hi, lo) fp16 pair with hi + lo ~= x."""
    hi = x.astype(np.float16)
    lo = (x - hi.astype(np.float32)).astype(np.float16)
    return hi, lo


def _make_in_maps(pred, target):
    in_maps = []
    for c in range(N_CORES):
        b, h = divmod(c, 2)
        p = pred[b, h * NH : (h + 1) * NH]  # [4096, 3]
        t = target[b]  # [8192, 3]
        pn = -0.5 * (p * p).sum(-1, dtype=np.float32)
        tn = -0.5 * (t * t).sum(-1, dtype=np.float32)
        ph, pl = _split16(p.T)
        th, tl = _split16(t.T)
        pnh, pnl = _split16(pn)
        tnh, tnl = _split16(tn)
        paug = np.zeros((K, NH), np.float16)
        taug = np.zeros((K, M), np.float16)
        # p.t = ph.th + pl.th + ph.tl ; norms via ones-rows
        paug[0:3] = ph
        paug[3:6] = pl
        paug[6:9] = ph
        paug[9] = pnh
        paug[10] = pnl
        paug[11] = 1.0
        paug[12] = 1.0
        taug[0:3] = th
        taug[3:6] = th
        taug[6:9] = tl
        taug[9] = 1.0
        taug[10] = 1.0
        taug[11] = tnh
        taug[12] = tnl
        in_maps.append({"paug": paug, "taug": taug})
    return in_maps


def _reduce_outputs(results):
    total = 0.0
    for b in range(B):
        fwd_rows = []
        bwd_parts = []
        for h in range(2):
            r = results[2 * b + h]
            fwd = np.asarray(r["fwd_out"], np.float64)  # [128, 32]
            fwd_rows.append(fwd.T.reshape(-1))  # row order n = t*128 + p
            bwd_parts.append(np.asarray(r["bwd_out"], np.float64).max(0))
        fwd_max = np.concatenate(fwd_rows)  # [8192]
        bwd_max = np.maximum(bwd_parts[0], bwd_parts[1])  # [8192]
        fwd_sq = np.maximum(-2.0 * fwd_max, EPS)
        bwd_sq = np.maximum(-2.0 * bwd_max, EPS)
        total += np.sqrt(fwd_sq).sum() + np.sqrt(bwd_sq).sum()
    return np.asarray(total / B, dtype=np.float32)


def kernel(pred, target):
    pred = np.ascontiguousarray(np.asarray(pred, dtype=np.float32))
    target = np.ascontiguousarray(np.asarray(target, dtype=np.float32))
    assert pred.shape == (B, N, D) and target.shape == (B, M, D)
    nc = _get_nc()
    in_maps = _make_in_maps(pred, target)
    res = run_bass_kernel_spmd(nc, in_maps, list(range(N_CORES)))
    return _reduce_outputs(res.results)
